# revision 7
# baseline (speedup 1.0000x reference)
"""Mamba-1 block (LN -> in_proj -> causal dwconv -> selective scan -> gated
out_proj) on 8 Trainium2 NeuronCores, tensor-parallel over d_inner.

Self-contained: hardcodes shapes from the problem spec.
  x:(2,2048,1024) in_proj_w:(4096,1024) conv_w:(2048,1,4) conv_b:(2048,)
  x_proj_w:(96,2048) dt_proj_w:(2048,64) dt_proj_b:(2048,) A_log:(2048,16)
  D:(2048,) out_proj_w:(1024,2048) norm_w:(1024,) norm_b:(1024,)

Per-core shard: 256 d_inner channels.  The only cross-core exchange is an
AllReduce of the x_proj output (96 rows) plus a host-side sum of the
out_proj partials.
"""
import numpy as np
import concourse.bacc as bacc
import concourse.tile as tile
from concourse import mybir
from concourse.bass_utils import run_bass_kernel_spmd

F32 = mybir.dt.float32
BF16 = mybir.dt.bfloat16
NPBF = mybir.dt.np(BF16)
AF = mybir.ActivationFunctionType
OP = mybir.AluOpType

D_MODEL = 1024
D_INNER = 2048
D_STATE = 16
DT_RANK = 64
B, L = 2, 2048
TOK = B * L            # 4096
CORES = 8
DC = D_INNER // CORES  # 256 channels/core
CH = 512               # token chunk for matmul/psum
NCH = TOK // CH        # 8
LH = 1024              # scan half-length
XD = DT_RANK + 2 * D_STATE  # 96

_cached = {}


def _build_nc(trace_label=None):
    nc = bacc.Bacc("TRN2", target_bir_lowering=False, debug=False,
                   num_devices=CORES)
    P = nc.declare_dram_parameter
    xt_e = P("xt", [D_MODEL, TOK], BF16, isOutput=False)
    w1t_e = P("w1t", [D_MODEL, 2 * DC], BF16, isOutput=False)
    k1_e = P("k1", [1, 2 * DC], BF16, isOutput=False)
    k2x_e = P("k2x", [128, 2], F32, isOutput=False)
    k2z_e = P("k2z", [128, 2], F32, isOutput=False)
    cdg_e = P("cdg", [128, 8 * 128], BF16, isOutput=False)
    cvb_e = P("cvb", [128, 2], F32, isOutput=False)
    wxt_e = P("wxt", [DC, XD], BF16, isOutput=False)
    dtw_e = P("dtw", [DT_RANK, DC], BF16, isOutput=False)
    dtb_e = P("dtb", [128, 2], F32, isOutput=False)
    aco_e = P("aco", [128, 32], F32, isOutput=False)
    dvc_e = P("dvc", [128, 2], F32, isOutput=False)
    wot_e = P("wot", [DC, D_MODEL], BF16, isOutput=False)
    idn_e = P("idn", [128, 128], BF16, isOutput=False)
    onc_e = P("onc", [128, 1], BF16, isOutput=False)
    out_e = P("outp", [D_MODEL, TOK], F32, isOutput=True)

    with tile.TileContext(nc) as tc:
        with tc.tile_pool(name="const", bufs=1) as cst, \
             tc.tile_pool(name="big", bufs=1) as big, \
             tc.tile_pool(name="xb", bufs=2) as xbp, \
             tc.tile_pool(name="sq", bufs=2) as sqp, \
             tc.tile_pool(name="sm", bufs=2) as smp, \
             tc.tile_pool(name="wk", bufs=2) as wkp, \
             tc.tile_pool(name="sc", bufs=2) as scp, \
             tc.tile_pool(name="bc", bufs=3) as bcp, \
             tc.tile_pool(name="ev", bufs=2) as evp, \
             tc.tile_pool(name="ps_st", bufs=1, space="PSUM") as ps_st, \
             tc.tile_pool(name="ps_mm", bufs=2, space="PSUM") as ps_mm, \
             tc.tile_pool(name="ps_y", bufs=1, space="PSUM") as ps_y, \
             tc.tile_pool(name="dram", bufs=2, space="DRAM") as drm:

            # ---- constants into SBUF ----
            w1t = cst.tile([128, 8 * 2 * DC], BF16, tag="w1t")  # f-major
            for f in range(8):
                nc.sync.dma_start(w1t[:, f * 512:(f + 1) * 512],
                                  w1t_e[f * 128:(f + 1) * 128, :])
            k1 = cst.tile([1, 2 * DC], BF16, tag="k1")
            nc.sync.dma_start(k1[:], k1_e[:])
            k2x = cst.tile([128, 2], F32, tag="k2x")
            nc.sync.dma_start(k2x[:], k2x_e[:])
            k2z = cst.tile([128, 2], F32, tag="k2z")
            nc.sync.dma_start(k2z[:], k2z_e[:])
            cdg = cst.tile([128, 8 * 128], BF16, tag="cdg")
            nc.sync.dma_start(cdg[:], cdg_e[:])
            cvb = cst.tile([128, 2], F32, tag="cvb")
            nc.sync.dma_start(cvb[:], cvb_e[:])
            wxt = cst.tile([128, 2 * XD], BF16, tag="wxt")
            for c in range(2):
                nc.sync.dma_start(wxt[:, c * XD:(c + 1) * XD],
                                  wxt_e[c * 128:(c + 1) * 128, :])
            dtw = cst.tile([DT_RANK, DC], BF16, tag="dtw")
            nc.sync.dma_start(dtw[:], dtw_e[:])
            dtb = cst.tile([128, 2], F32, tag="dtb")
            nc.sync.dma_start(dtb[:], dtb_e[:])
            aco = cst.tile([128, 32], F32, tag="aco")
            nc.sync.dma_start(aco[:], aco_e[:])
            dvc = cst.tile([128, 2], F32, tag="dvc")
            nc.sync.dma_start(dvc[:], dvc_e[:])
            wot = cst.tile([128, 2 * D_MODEL], BF16, tag="wot")
            for c in range(2):
                nc.sync.dma_start(wot[:, c * D_MODEL:(c + 1) * D_MODEL],
                                  wot_e[c * 128:(c + 1) * 128, :])
            idn = cst.tile([128, 128], BF16, tag="idn")
            nc.sync.dma_start(idn[:], idn_e[:])
            onc = cst.tile([128, 1], BF16, tag="onc")
            nc.sync.dma_start(onc[:], onc_e[:])
            eps = cst.tile([1, 1], F32, tag="eps")
            nc.vector.memset(eps[:], 1e-5)

            # ---- persistent activations ----
            xin = big.tile([128, 2 * TOK], BF16, tag="xin")   # cht-major
            zs = big.tile([128, 2 * TOK], BF16, tag="zs")     # z, later silu(z), later gated y
            u_sb = big.tile([128, 2 * TOK], BF16, tag="u")
            dt_sb = big.tile([128, 2 * TOK], BF16, tag="dt")
            du_sb = big.tile([128, 2 * TOK], BF16, tag="du")
            xdf = big.tile([XD, TOK], BF16, tag="xdf")
            carry = big.tile([128, 2 * D_STATE], F32, tag="carry")

            rstd_d = drm.tile([1, TOK], BF16, tag="rstd_d")
            bcr_d = drm.tile([2 * D_STATE, TOK], BF16, tag="bcr_d")

            # ================= phase A: LN stats + in_proj =================
            for c in range(NCH):
                t0 = c * CH
                xc = []
                for f in range(8):
                    xt_t = xbp.tile([128, CH], BF16, tag=f"xc{f}")
                    nc.sync.dma_start(xt_t[:],
                                      xt_e[f * 128:(f + 1) * 128, t0:t0 + CH])
                    xc.append(xt_t)
                # stats: sum(x) and sum(x^2) over features via PE
                psx = ps_st.tile([1, CH], F32, tag="psx")
                for f in range(8):
                    nc.tensor.matmul(psx[:], onc[:], xc[f][:],
                                     start=(f == 0), stop=(f == 7))
                pxx = ps_st.tile([1, CH], F32, tag="pxx")
                for f in range(8):
                    sq = sqp.tile([128, CH], BF16, tag="sq")
                    nc.scalar.activation(sq[:], xc[f][:], AF.Square)
                    nc.tensor.matmul(pxx[:], onc[:], sq[:],
                                     start=(f == 0), stop=(f == 7))
                # negmu, var, rstd = exp(-0.5*ln(var+eps))
                nmu_f = smp.tile([1, CH], F32, tag="nmu_f")
                nc.scalar.activation(nmu_f[:], psx[:], AF.Copy,
                                     scale=-1.0 / D_MODEL)
                nmu_b = smp.tile([1, CH], BF16, tag="nmu_b")
                nc.scalar.activation(nmu_b[:], psx[:], AF.Copy,
                                     scale=-1.0 / D_MODEL)
                s2 = smp.tile([1, CH], F32, tag="s2")
                nc.scalar.activation(s2[:], pxx[:], AF.Copy,
                                     scale=1.0 / D_MODEL)
                msq = smp.tile([1, CH], F32, tag="msq")
                nc.scalar.activation(msq[:], nmu_f[:], AF.Square)
                var = smp.tile([1, CH], F32, tag="var")
                nc.vector.tensor_sub(var[:], s2[:], msq[:])
                nc.scalar.activation(var[:], var[:], AF.Ln, bias=eps[:])
                rst = smp.tile([1, CH], BF16, tag="rst")
                nc.scalar.activation(rst[:], var[:], AF.Exp, scale=-0.5)
                nc.sync.dma_start(rstd_d[:, t0:t0 + CH], rst[:])
                rbc = wkp.tile([128, CH], BF16, tag="rbc")
                nc.sync.dma_start(
                    rbc[:], rstd_d[0:1, t0:t0 + CH].broadcast_to([128, CH]))
                # in_proj (4 output ch-tiles: xin cht0/1, z cht0/1)
                for m in range(4):
                    pxz = ps_mm.tile([128, CH], F32, tag="mm")
                    for f in range(8):
                        nc.tensor.matmul(
                            pxz[:], w1t[:, f * 512 + m * 128:f * 512 + (m + 1) * 128],
                            xc[f][:], start=(f == 0), stop=False)
                    nc.tensor.matmul(pxz[:], k1[0:1, m * 128:(m + 1) * 128],
                                     nmu_b[:], start=False, stop=True)
                    if m < 2:
                        dst = xin[:, m * TOK + t0: m * TOK + t0 + CH]
                        nc.vector.tensor_mul(dst, pxz[:], rbc[:])
                        nc.vector.tensor_scalar_add(dst, dst, k2x[:, m:m + 1])
                    else:
                        dst = zs[:, (m - 2) * TOK + t0:(m - 2) * TOK + t0 + CH]
                        nc.vector.tensor_mul(dst, pxz[:], rbc[:])
            # z -> silu(z + k2z) in place
            for cht in range(2):
                nc.scalar.activation(zs[:, cht * TOK:(cht + 1) * TOK],
                                     zs[:, cht * TOK:(cht + 1) * TOK],
                                     AF.Silu, bias=k2z[:, cht:cht + 1])

            # ============== phase B: conv + silu + x_proj (+AR) ============
            for b in range(B):
                for cht in range(2):
                    for c4 in range(4):
                        base = cht * TOK + b * L + c4 * CH
                        pcv = ps_mm.tile([128, CH], F32, tag="mm")
                        nc.tensor.matmul(
                            pcv[:], cdg[:, (cht * 4 + 3) * 128:(cht * 4 + 4) * 128],
                            xin[:, base:base + CH], start=True, stop=False)
                        for j in range(3):
                            sh = 3 - j
                            wsl = cdg[:, (cht * 4 + j) * 128:(cht * 4 + j + 1) * 128]
                            if c4 == 0:
                                nc.tensor.matmul(pcv[:, sh:CH], wsl,
                                                 xin[:, base:base + CH - sh],
                                                 start=False, stop=(j == 2))
                            else:
                                nc.tensor.matmul(pcv[:], wsl,
                                                 xin[:, base - sh:base - sh + CH],
                                                 start=False, stop=(j == 2))
                        nc.scalar.activation(
                            u_sb[:, base:base + CH], pcv[:], AF.Silu,
                            bias=cvb[:, cht:cht + 1])
                for c4 in range(4):
                    t0 = b * L + c4 * CH
                    pxd = ps_mm.tile([XD, CH], F32, tag="mm")
                    for cht in range(2):
                        nc.tensor.matmul(pxd[:], wxt[:, cht * XD:(cht + 1) * XD],
                                         u_sb[:, cht * TOK + t0:cht * TOK + t0 + CH],
                                         start=(cht == 0), stop=(cht == 1))
                    nc.scalar.copy(xdf[:, t0:t0 + CH], pxd[:])
                # all-reduce this batch's x_dbl
                ari = drm.tile([XD, L], BF16, tag="ari")
                aro = drm.tile([XD, L], BF16, tag="aro")
                nc.gpsimd.dma_start(ari[:], xdf[:, b * L:(b + 1) * L])
                nc.gpsimd.collective_compute(
                    "AllReduce", OP.add,
                    replica_groups=[list(range(CORES))],
                    ins=[ari.opt()], outs=[aro.opt()])
                nc.gpsimd.dma_start(xdf[:, b * L:(b + 1) * L], aro[:])

                # ---- phase C: dt_proj + softplus + du; stage B/C rows ----
                nc.sync.dma_start(bcr_d[:, b * L:(b + 1) * L],
                                  xdf[DT_RANK:XD, b * L:(b + 1) * L])
                for cht in range(2):
                    for c4 in range(4):
                        t0 = b * L + c4 * CH
                        pdt = ps_mm.tile([128, CH], F32, tag="mm")
                        nc.tensor.matmul(pdt[:], dtw[:, cht * 128:(cht + 1) * 128],
                                         xdf[0:DT_RANK, t0:t0 + CH],
                                         start=True, stop=True)
                        pt = wkp.tile([128, CH], F32, tag="pt")
                        nc.scalar.activation(pt[:], pdt[:], AF.Exp,
                                             bias=dtb[:, cht:cht + 1])
                        nc.scalar.activation(
                            dt_sb[:, cht * TOK + t0:cht * TOK + t0 + CH],
                            pt[:], AF.Ln, bias=1.0)
                    off = cht * TOK + b * L
                    nc.vector.tensor_mul(du_sb[:, off:off + L],
                                         dt_sb[:, off:off + L],
                                         u_sb[:, off:off + L])

                # ---- phase D: selective scan (half-L chunks) ----
                for lh in range(2):
                    toff = b * L + lh * LH
                    pys = [ps_y.tile([128, LH], F32, tag=f"py{c}",
                                     name=f"py{c}") for c in range(2)]
                    for s in range(D_STATE):
                        bbc = bcp.tile([128, LH], BF16, tag="bbc")
                        nc.sync.dma_start(
                            bbc[:], bcr_d[s:s + 1, toff:toff + LH]
                            .broadcast_to([128, LH]))
                        cbc = bcp.tile([128, LH], BF16, tag="cbc")
                        nc.gpsimd.dma_start(
                            cbc[:], bcr_d[D_STATE + s:D_STATE + s + 1,
                                          toff:toff + LH]
                            .broadcast_to([128, LH]))
                        for cht in range(2):
                            off = cht * TOK + toff
                            dA = scp.tile([128, LH], F32, tag="dA")
                            nc.scalar.activation(
                                dA[:], dt_sb[:, off:off + LH], AF.Exp,
                                scale=aco[:, cht * 16 + s:cht * 16 + s + 1])
                            bt = scp.tile([128, LH], BF16, tag="bt")
                            nc.vector.tensor_mul(bt[:], du_sb[:, off:off + LH],
                                                 bbc[:])
                            h = scp.tile([128, LH], BF16, tag="h")
                            init = (0.0 if lh == 0 else
                                    carry[:, cht * D_STATE + s:
                                          cht * D_STATE + s + 1])
                            nc.vector.tensor_tensor_scan(
                                h[:], dA[:], bt[:], init,
                                op0=OP.mult, op1=OP.add)
                            if lh == 0:
                                nc.vector.tensor_copy(
                                    carry[:, cht * D_STATE + s:
                                          cht * D_STATE + s + 1],
                                    h[:, LH - 1:LH])
                            nc.vector.tensor_mul(h[:], h[:], cbc[:])
                            for q in range(2):
                                nc.tensor.matmul(
                                    pys[cht][:, q * CH:(q + 1) * CH], idn[:],
                                    h[:, q * CH:(q + 1) * CH],
                                    start=(s == 0), stop=(s == D_STATE - 1))
                    # ---- phase E: skip + gate ----
                    for cht in range(2):
                        off = cht * TOK + toff
                        yd = evp.tile([128, LH], BF16, tag="yd")
                        nc.vector.scalar_tensor_tensor(
                            yd[:], u_sb[:, off:off + LH], dvc[:, cht:cht + 1],
                            pys[cht][:], op0=OP.mult, op1=OP.add)
                        nc.vector.tensor_mul(zs[:, off:off + LH], yd[:],
                                             zs[:, off:off + LH])

                # ---- phase F: out_proj partial for this batch ----
                for e in range(8):
                    for c4 in range(4):
                        t0 = b * L + c4 * CH
                        po = ps_mm.tile([128, CH], F32, tag="mm")
                        for cht in range(2):
                            nc.tensor.matmul(
                                po[:], wot[:, cht * D_MODEL + e * 128:
                                           cht * D_MODEL + (e + 1) * 128],
                                zs[:, cht * TOK + t0:cht * TOK + t0 + CH],
                                start=(cht == 0), stop=(cht == 1))
                        ob = evp.tile([128, CH], F32, tag="ob")
                        nc.scalar.copy(ob[:], po[:])
                        nc.sync.dma_start(
                            out_e[e * 128:(e + 1) * 128, t0:t0 + CH], ob[:])
    nc.finalize()
    return nc


def _host_prep(inputs, k):
    x = np.asarray(inputs["x"], np.float32)
    ipw = np.asarray(inputs["in_proj_w"], np.float32)
    cw = np.asarray(inputs["conv_w"], np.float32)
    cb = np.asarray(inputs["conv_b"], np.float32)
    xpw = np.asarray(inputs["x_proj_w"], np.float32)
    dpw = np.asarray(inputs["dt_proj_w"], np.float32)
    dpb = np.asarray(inputs["dt_proj_b"], np.float32)
    alog = np.asarray(inputs["A_log"], np.float32)
    dd = np.asarray(inputs["D"], np.float32)
    opw = np.asarray(inputs["out_proj_w"], np.float32)
    nw = np.asarray(inputs["norm_w"], np.float32)
    nb = np.asarray(inputs["norm_b"], np.float32)

    sl = slice(k * DC, (k + 1) * DC)
    w1s = np.concatenate([ipw[sl], ipw[D_INNER + k * DC:D_INNER + (k + 1) * DC]], 0)
    w1w = w1s * nw[None, :]

    def col2(v):  # (256,) -> (128, 2), col = cht
        return np.ascontiguousarray(v.reshape(2, 128).T, dtype=np.float32)

    cdg = np.zeros((128, 8 * 128), NPBF)
    cwk = cw[sl, 0, :]  # (256, 4)
    for cht in range(2):
        for j in range(4):
            blk = np.diag(cwk[cht * 128:(cht + 1) * 128, j]).astype(NPBF)
            cdg[:, (cht * 4 + j) * 128:(cht * 4 + j + 1) * 128] = blk

    acol = (-np.exp(alog[sl])).astype(np.float32)  # (256, 16)
    acol = np.ascontiguousarray(
        acol.reshape(2, 128, D_STATE).transpose(1, 0, 2).reshape(128, 32))

    return {
        "xt": np.ascontiguousarray(
            x.reshape(TOK, D_MODEL).T).astype(NPBF),
        "w1t": np.ascontiguousarray(w1w.T).astype(NPBF),
        "k1": w1w.sum(1).astype(NPBF).reshape(1, 2 * DC),
        "k2x": col2((w1s[:DC] @ nb)),
        "k2z": col2((w1s[DC:] @ nb)),
        "cdg": cdg,
        "cvb": col2(cb[sl]),
        "wxt": np.ascontiguousarray(xpw[:, sl].T).astype(NPBF),
        "dtw": np.ascontiguousarray(dpw[sl].T).astype(NPBF),
        "dtb": col2(dpb[sl]),
        "aco": acol,
        "dvc": col2(dd[sl]),
        "wot": np.ascontiguousarray(opw[:, sl].T).astype(NPBF),
        "idn": np.eye(128, dtype=NPBF),
        "onc": np.ones((128, 1), NPBF),
    }


def kernel(**inputs):
    if "nc" not in _cached:
        _cached["nc"] = _build_nc()
    nc = _cached["nc"]
    in_maps = [_host_prep(inputs, k) for k in range(CORES)]
    res = run_bass_kernel_spmd(nc, in_maps, list(range(CORES)))
    _cached["last_result"] = res
    out = np.zeros((D_MODEL, TOK), np.float64)
    for k in range(CORES):
        out += res.results[k]["outp"]
    out = out.astype(np.float32).T.reshape(B, L, D_MODEL)
    residual = np.asarray(inputs["x"], np.float32)
    return out, residual


# revision 9
# speedup vs baseline: 1.0576x; 1.0576x over previous
"""Mamba-1 block (LN -> in_proj -> causal dwconv -> selective scan -> gated
out_proj) on 8 Trainium2 NeuronCores, tensor-parallel over d_inner.

Self-contained: hardcodes shapes from the problem spec.
  x:(2,2048,1024) in_proj_w:(4096,1024) conv_w:(2048,1,4) conv_b:(2048,)
  x_proj_w:(96,2048) dt_proj_w:(2048,64) dt_proj_b:(2048,) A_log:(2048,16)
  D:(2048,) out_proj_w:(1024,2048) norm_w:(1024,) norm_b:(1024,)

Per-core shard: 256 d_inner channels.  The only cross-core exchange is an
AllReduce of the x_proj output (96 rows) plus a host-side sum of the
out_proj partials.
"""
import numpy as np
import concourse.bacc as bacc
import concourse.tile as tile
from concourse import mybir
from concourse.bass_utils import run_bass_kernel_spmd

F32 = mybir.dt.float32
BF16 = mybir.dt.bfloat16
NPBF = mybir.dt.np(BF16)
AF = mybir.ActivationFunctionType
OP = mybir.AluOpType

D_MODEL = 1024
D_INNER = 2048
D_STATE = 16
DT_RANK = 64
B, L = 2, 2048
TOK = B * L            # 4096
CORES = 8
DC = D_INNER // CORES  # 256 channels/core
CH = 512               # token chunk for matmul/psum
NCH = TOK // CH        # 8
LH = 1024              # scan half-length
XD = DT_RANK + 2 * D_STATE  # 96

_cached = {}


def _build_nc(trace_label=None):
    nc = bacc.Bacc("TRN2", target_bir_lowering=False, debug=False,
                   num_devices=CORES)
    P = nc.declare_dram_parameter
    xt_e = P("xt", [D_MODEL, TOK], BF16, isOutput=False)
    w1t_e = P("w1t", [D_MODEL, 2 * DC], BF16, isOutput=False)
    k1_e = P("k1", [1, 2 * DC], BF16, isOutput=False)
    k2x_e = P("k2x", [128, 2], F32, isOutput=False)
    k2z_e = P("k2z", [128, 2], F32, isOutput=False)
    cdg_e = P("cdg", [128, 8 * 128], BF16, isOutput=False)
    cvb_e = P("cvb", [128, 2], F32, isOutput=False)
    wxt_e = P("wxt", [DC, XD], BF16, isOutput=False)
    dtw_e = P("dtw", [DT_RANK, DC], BF16, isOutput=False)
    dtb_e = P("dtb", [128, 2], F32, isOutput=False)
    aco_e = P("aco", [128, 32], F32, isOutput=False)
    dvc_e = P("dvc", [128, 2], F32, isOutput=False)
    wot_e = P("wot", [DC, D_MODEL], BF16, isOutput=False)
    idn_e = P("idn", [128, 128], BF16, isOutput=False)
    onc_e = P("onc", [128, 1], BF16, isOutput=False)
    out_e = P("outp", [D_MODEL, TOK], F32, isOutput=True)

    with tile.TileContext(nc) as tc:
        with tc.tile_pool(name="const", bufs=1) as cst, \
             tc.tile_pool(name="big", bufs=1) as big, \
             tc.tile_pool(name="xb", bufs=2) as xbp, \
             tc.tile_pool(name="sq", bufs=2) as sqp, \
             tc.tile_pool(name="sm", bufs=2) as smp, \
             tc.tile_pool(name="wk", bufs=2) as wkp, \
             tc.tile_pool(name="sc", bufs=2) as scp, \
             tc.tile_pool(name="bc", bufs=6) as bcp, \
             tc.tile_pool(name="ev", bufs=2) as evp, \
             tc.tile_pool(name="ps_st", bufs=1, space="PSUM") as ps_st, \
             tc.tile_pool(name="ps_mm", bufs=2, space="PSUM") as ps_mm, \
             tc.tile_pool(name="ps_y", bufs=1, space="PSUM") as ps_y, \
             tc.tile_pool(name="dram", bufs=2, space="DRAM") as drm:

            # ---- constants into SBUF ----
            w1t = cst.tile([128, 8 * 2 * DC], BF16, tag="w1t")  # f-major
            for f in range(8):
                nc.sync.dma_start(w1t[:, f * 512:(f + 1) * 512],
                                  w1t_e[f * 128:(f + 1) * 128, :])
            k1 = cst.tile([1, 2 * DC], BF16, tag="k1")
            nc.sync.dma_start(k1[:], k1_e[:])
            k2x = cst.tile([128, 2], F32, tag="k2x")
            nc.sync.dma_start(k2x[:], k2x_e[:])
            k2z = cst.tile([128, 2], F32, tag="k2z")
            nc.sync.dma_start(k2z[:], k2z_e[:])
            cdg = cst.tile([128, 8 * 128], BF16, tag="cdg")
            nc.sync.dma_start(cdg[:], cdg_e[:])
            cvb = cst.tile([128, 2], F32, tag="cvb")
            nc.sync.dma_start(cvb[:], cvb_e[:])
            wxt = cst.tile([128, 2 * XD], BF16, tag="wxt")
            for c in range(2):
                nc.sync.dma_start(wxt[:, c * XD:(c + 1) * XD],
                                  wxt_e[c * 128:(c + 1) * 128, :])
            dtw = cst.tile([DT_RANK, DC], BF16, tag="dtw")
            nc.sync.dma_start(dtw[:], dtw_e[:])
            dtb = cst.tile([128, 2], F32, tag="dtb")
            nc.sync.dma_start(dtb[:], dtb_e[:])
            aco = cst.tile([128, 32], F32, tag="aco")
            nc.sync.dma_start(aco[:], aco_e[:])
            dvc = cst.tile([128, 2], F32, tag="dvc")
            nc.sync.dma_start(dvc[:], dvc_e[:])
            wot = cst.tile([128, 2 * D_MODEL], BF16, tag="wot")
            for c in range(2):
                nc.sync.dma_start(wot[:, c * D_MODEL:(c + 1) * D_MODEL],
                                  wot_e[c * 128:(c + 1) * 128, :])
            idn = cst.tile([128, 128], BF16, tag="idn")
            nc.sync.dma_start(idn[:], idn_e[:])
            onc = cst.tile([128, 1], BF16, tag="onc")
            nc.sync.dma_start(onc[:], onc_e[:])
            eps = cst.tile([1, 1], F32, tag="eps")
            nc.vector.memset(eps[:], 1e-5)

            # ---- persistent activations ----
            xin = big.tile([128, 2 * TOK], BF16, tag="xin")   # cht-major
            zs = big.tile([128, 2 * TOK], BF16, tag="zs")     # z, later silu(z), later gated y
            u_sb = big.tile([128, 2 * TOK], BF16, tag="u")
            dt_sb = big.tile([128, 2 * TOK], BF16, tag="dt")
            du_sb = big.tile([128, 2 * TOK], BF16, tag="du")
            xdf = big.tile([XD, TOK], BF16, tag="xdf")
            carry = big.tile([128, 2 * D_STATE], F32, tag="carry")

            rstd_d = drm.tile([1, TOK], BF16, tag="rstd_d")
            bcr_d = drm.tile([2 * D_STATE, TOK], BF16, tag="bcr_d")

            # ================= phase A: LN stats + in_proj =================
            for c in range(NCH):
                t0 = c * CH
                xc = []
                for f in range(8):
                    xt_t = xbp.tile([128, CH], BF16, tag=f"xc{f}")
                    nc.sync.dma_start(xt_t[:],
                                      xt_e[f * 128:(f + 1) * 128, t0:t0 + CH])
                    xc.append(xt_t)
                # stats: sum(x) and sum(x^2) over features via PE
                psx = ps_st.tile([1, CH], F32, tag="psx")
                for f in range(8):
                    nc.tensor.matmul(psx[:], onc[:], xc[f][:],
                                     start=(f == 0), stop=(f == 7))
                pxx = ps_st.tile([1, CH], F32, tag="pxx")
                for f in range(8):
                    sq = sqp.tile([128, CH], BF16, tag="sq")
                    nc.scalar.activation(sq[:], xc[f][:], AF.Square)
                    nc.tensor.matmul(pxx[:], onc[:], sq[:],
                                     start=(f == 0), stop=(f == 7))
                # negmu, var, rstd = exp(-0.5*ln(var+eps))
                nmu_f = smp.tile([1, CH], F32, tag="nmu_f")
                nc.scalar.activation(nmu_f[:], psx[:], AF.Copy,
                                     scale=-1.0 / D_MODEL)
                nmu_b = smp.tile([1, CH], BF16, tag="nmu_b")
                nc.scalar.activation(nmu_b[:], psx[:], AF.Copy,
                                     scale=-1.0 / D_MODEL)
                s2 = smp.tile([1, CH], F32, tag="s2")
                nc.scalar.activation(s2[:], pxx[:], AF.Copy,
                                     scale=1.0 / D_MODEL)
                msq = smp.tile([1, CH], F32, tag="msq")
                nc.scalar.activation(msq[:], nmu_f[:], AF.Square)
                var = smp.tile([1, CH], F32, tag="var")
                nc.vector.tensor_sub(var[:], s2[:], msq[:])
                nc.scalar.activation(var[:], var[:], AF.Ln, bias=eps[:])
                rst = smp.tile([1, CH], BF16, tag="rst")
                nc.scalar.activation(rst[:], var[:], AF.Exp, scale=-0.5)
                nc.sync.dma_start(rstd_d[:, t0:t0 + CH], rst[:])
                rbc = wkp.tile([128, CH], BF16, tag="rbc")
                nc.sync.dma_start(
                    rbc[:], rstd_d[0:1, t0:t0 + CH].broadcast_to([128, CH]))
                # in_proj (4 output ch-tiles: xin cht0/1, z cht0/1)
                for m in range(4):
                    pxz = ps_mm.tile([128, CH], F32, tag="mm")
                    for f in range(8):
                        nc.tensor.matmul(
                            pxz[:], w1t[:, f * 512 + m * 128:f * 512 + (m + 1) * 128],
                            xc[f][:], start=(f == 0), stop=False)
                    nc.tensor.matmul(pxz[:], k1[0:1, m * 128:(m + 1) * 128],
                                     nmu_b[:], start=False, stop=True)
                    if m < 2:
                        dst = xin[:, m * TOK + t0: m * TOK + t0 + CH]
                        nc.vector.tensor_mul(dst, pxz[:], rbc[:])
                        nc.vector.tensor_scalar_add(dst, dst, k2x[:, m:m + 1])
                    else:
                        dst = zs[:, (m - 2) * TOK + t0:(m - 2) * TOK + t0 + CH]
                        nc.vector.tensor_mul(dst, pxz[:], rbc[:])
            # z -> silu(z + k2z) in place
            for cht in range(2):
                nc.scalar.activation(zs[:, cht * TOK:(cht + 1) * TOK],
                                     zs[:, cht * TOK:(cht + 1) * TOK],
                                     AF.Silu, bias=k2z[:, cht:cht + 1])

            # ============== phases B..F as emit-order closures ============
            def phase_B(b):
                for cht in range(2):
                    for c4 in range(4):
                        base = cht * TOK + b * L + c4 * CH
                        pcv = ps_mm.tile([128, CH], F32, tag="mm", name="pcv")
                        nc.tensor.matmul(
                            pcv[:], cdg[:, (cht * 4 + 3) * 128:(cht * 4 + 4) * 128],
                            xin[:, base:base + CH], start=True, stop=False)
                        for j in range(3):
                            sh = 3 - j
                            wsl = cdg[:, (cht * 4 + j) * 128:(cht * 4 + j + 1) * 128]
                            if c4 == 0:
                                nc.tensor.matmul(pcv[:, sh:CH], wsl,
                                                 xin[:, base:base + CH - sh],
                                                 start=False, stop=(j == 2))
                            else:
                                nc.tensor.matmul(pcv[:], wsl,
                                                 xin[:, base - sh:base - sh + CH],
                                                 start=False, stop=(j == 2))
                        nc.scalar.activation(
                            u_sb[:, base:base + CH], pcv[:], AF.Silu,
                            bias=cvb[:, cht:cht + 1])
                for c4 in range(4):
                    t0 = b * L + c4 * CH
                    pxd = ps_mm.tile([XD, CH], F32, tag="mm", name="pxd")
                    for cht in range(2):
                        nc.tensor.matmul(pxd[:], wxt[:, cht * XD:(cht + 1) * XD],
                                         u_sb[:, cht * TOK + t0:cht * TOK + t0 + CH],
                                         start=(cht == 0), stop=(cht == 1))
                    nc.scalar.copy(xdf[:, t0:t0 + CH], pxd[:])

            def phase_AR(b):
                ari = drm.tile([XD, L], BF16, tag="ari", name="ari")
                aro = drm.tile([XD, L], BF16, tag="aro", name="aro")
                nc.sync.dma_start(ari[:], xdf[:, b * L:(b + 1) * L])
                nc.gpsimd.collective_compute(
                    "AllReduce", OP.add,
                    replica_groups=[list(range(CORES))],
                    ins=[ari.opt()], outs=[aro.opt()])
                nc.sync.dma_start(xdf[:, b * L:(b + 1) * L], aro[:])

            def phase_C(b):
                nc.sync.dma_start(bcr_d[:, b * L:(b + 1) * L],
                                  xdf[DT_RANK:XD, b * L:(b + 1) * L])
                for cht in range(2):
                    for c4 in range(4):
                        t0 = b * L + c4 * CH
                        pdt = ps_mm.tile([128, CH], F32, tag="mm", name="pdt")
                        nc.tensor.matmul(pdt[:], dtw[:, cht * 128:(cht + 1) * 128],
                                         xdf[0:DT_RANK, t0:t0 + CH],
                                         start=True, stop=True)
                        pt = wkp.tile([128, CH], F32, tag="pt", name="pt")
                        nc.scalar.activation(pt[:], pdt[:], AF.Exp,
                                             bias=dtb[:, cht:cht + 1])
                        nc.scalar.activation(
                            dt_sb[:, cht * TOK + t0:cht * TOK + t0 + CH],
                            pt[:], AF.Ln, bias=1.0)
                    off = cht * TOK + b * L
                    nc.vector.tensor_mul(du_sb[:, off:off + L],
                                         dt_sb[:, off:off + L],
                                         u_sb[:, off:off + L])

            def phase_DE(b):
                for lh in range(2):
                    toff = b * L + lh * LH
                    pys = [ps_y.tile([128, LH], F32, tag=f"py{c}",
                                     name=f"py{c}") for c in range(2)]
                    for s in range(D_STATE):
                        bbc = bcp.tile([128, LH], BF16, tag="bbc", name="bbc")
                        nc.sync.dma_start(
                            bbc[:], bcr_d[s:s + 1, toff:toff + LH]
                            .broadcast_to([128, LH]))
                        cbc = bcp.tile([128, LH], BF16, tag="cbc", name="cbc")
                        nc.scalar.dma_start(
                            cbc[:], bcr_d[D_STATE + s:D_STATE + s + 1,
                                          toff:toff + LH]
                            .broadcast_to([128, LH]))
                        for cht in range(2):
                            off = cht * TOK + toff
                            dA = scp.tile([128, LH], F32, tag="dA", name="dA")
                            nc.scalar.activation(
                                dA[:], dt_sb[:, off:off + LH], AF.Exp,
                                scale=aco[:, cht * 16 + s:cht * 16 + s + 1])
                            bt = scp.tile([128, LH], BF16, tag="bt", name="bt")
                            nc.vector.tensor_mul(bt[:], du_sb[:, off:off + LH],
                                                 bbc[:])
                            h = scp.tile([128, LH], BF16, tag="h", name="h")
                            init = (0.0 if lh == 0 else
                                    carry[:, cht * D_STATE + s:
                                          cht * D_STATE + s + 1])
                            nc.vector.tensor_tensor_scan(
                                h[:], dA[:], bt[:], init,
                                op0=OP.mult, op1=OP.add)
                            if lh == 0:
                                nc.vector.tensor_copy(
                                    carry[:, cht * D_STATE + s:
                                          cht * D_STATE + s + 1],
                                    h[:, LH - 1:LH])
                            nc.vector.tensor_mul(h[:], h[:], cbc[:])
                            for q in range(2):
                                nc.tensor.matmul(
                                    pys[cht][:, q * CH:(q + 1) * CH], idn[:],
                                    h[:, q * CH:(q + 1) * CH],
                                    start=(s == 0), stop=(s == D_STATE - 1))
                    for cht in range(2):
                        off = cht * TOK + toff
                        yd = evp.tile([128, LH], BF16, tag="yd", name="yd")
                        nc.vector.scalar_tensor_tensor(
                            yd[:], u_sb[:, off:off + LH], dvc[:, cht:cht + 1],
                            pys[cht][:], op0=OP.mult, op1=OP.add)
                        nc.vector.tensor_mul(zs[:, off:off + LH], yd[:],
                                             zs[:, off:off + LH])

            def phase_F(b):
                for e in range(8):
                    for c4 in range(4):
                        t0 = b * L + c4 * CH
                        po = ps_mm.tile([128, CH], F32, tag="mm", name="po")
                        for cht in range(2):
                            nc.tensor.matmul(
                                po[:], wot[:, cht * D_MODEL + e * 128:
                                           cht * D_MODEL + (e + 1) * 128],
                                zs[:, cht * TOK + t0:cht * TOK + t0 + CH],
                                start=(cht == 0), stop=(cht == 1))
                        ob = evp.tile([128, CH], F32, tag="ob", name="ob")
                        nc.scalar.copy(ob[:], po[:])
                        nc.sync.dma_start(
                            out_e[e * 128:(e + 1) * 128, t0:t0 + CH], ob[:])

            phase_B(0)
            phase_AR(0)
            phase_B(1)
            phase_AR(1)
            phase_C(0)
            phase_C(1)
            phase_DE(0)
            phase_F(0)
            phase_DE(1)
            phase_F(1)
    nc.finalize()
    return nc


def _host_prep(inputs, k):
    x = np.asarray(inputs["x"], np.float32)
    ipw = np.asarray(inputs["in_proj_w"], np.float32)
    cw = np.asarray(inputs["conv_w"], np.float32)
    cb = np.asarray(inputs["conv_b"], np.float32)
    xpw = np.asarray(inputs["x_proj_w"], np.float32)
    dpw = np.asarray(inputs["dt_proj_w"], np.float32)
    dpb = np.asarray(inputs["dt_proj_b"], np.float32)
    alog = np.asarray(inputs["A_log"], np.float32)
    dd = np.asarray(inputs["D"], np.float32)
    opw = np.asarray(inputs["out_proj_w"], np.float32)
    nw = np.asarray(inputs["norm_w"], np.float32)
    nb = np.asarray(inputs["norm_b"], np.float32)

    sl = slice(k * DC, (k + 1) * DC)
    w1s = np.concatenate([ipw[sl], ipw[D_INNER + k * DC:D_INNER + (k + 1) * DC]], 0)
    w1w = w1s * nw[None, :]

    def col2(v):  # (256,) -> (128, 2), col = cht
        return np.ascontiguousarray(v.reshape(2, 128).T, dtype=np.float32)

    cdg = np.zeros((128, 8 * 128), NPBF)
    cwk = cw[sl, 0, :]  # (256, 4)
    for cht in range(2):
        for j in range(4):
            blk = np.diag(cwk[cht * 128:(cht + 1) * 128, j]).astype(NPBF)
            cdg[:, (cht * 4 + j) * 128:(cht * 4 + j + 1) * 128] = blk

    acol = (-np.exp(alog[sl])).astype(np.float32)  # (256, 16)
    acol = np.ascontiguousarray(
        acol.reshape(2, 128, D_STATE).transpose(1, 0, 2).reshape(128, 32))

    return {
        "xt": np.ascontiguousarray(
            x.reshape(TOK, D_MODEL).T).astype(NPBF),
        "w1t": np.ascontiguousarray(w1w.T).astype(NPBF),
        "k1": w1w.sum(1).astype(NPBF).reshape(1, 2 * DC),
        "k2x": col2((w1s[:DC] @ nb)),
        "k2z": col2((w1s[DC:] @ nb)),
        "cdg": cdg,
        "cvb": col2(cb[sl]),
        "wxt": np.ascontiguousarray(xpw[:, sl].T).astype(NPBF),
        "dtw": np.ascontiguousarray(dpw[sl].T).astype(NPBF),
        "dtb": col2(dpb[sl]),
        "aco": acol,
        "dvc": col2(dd[sl]),
        "wot": np.ascontiguousarray(opw[:, sl].T).astype(NPBF),
        "idn": np.eye(128, dtype=NPBF),
        "onc": np.ones((128, 1), NPBF),
    }


def kernel(**inputs):
    if "nc" not in _cached:
        _cached["nc"] = _build_nc()
    nc = _cached["nc"]
    in_maps = [_host_prep(inputs, k) for k in range(CORES)]
    res = run_bass_kernel_spmd(nc, in_maps, list(range(CORES)))
    _cached["last_result"] = res
    out = np.zeros((D_MODEL, TOK), np.float64)
    for k in range(CORES):
        out += res.results[k]["outp"]
    out = out.astype(np.float32).T.reshape(B, L, D_MODEL)
    residual = np.asarray(inputs["x"], np.float32)
    return out, residual
